# revision 30
# baseline (speedup 1.0000x reference)
"""Trainium2 Bass kernel for ConfigurableNoisyQuantumLayer.

Math: the circuit is a fixed sequence of single-qubit rotations, CNOTs and
noise channels acting on an 8-qubit density matrix, batched over 32 inputs x.
In the (real) Pauli-transfer-matrix picture every channel is a real 4^n x 4^n
matrix. We pull the observable Z_0 back through the 6 layers (Heisenberg
picture) -- one shared real (4^4)x(4^4)=256x256 matrix chain independent of
the batch -- then contract with per-sample product-state Pauli vectors.

Per adjoint layer (l = 5..0), with q the 256x256 pullback matrix
(rows = wires 0-3 pair-index, cols = wires 4-7):
    T  = A_l q          A_l = kron_{w=0..3} F(l,w)^T
    W  = T B_l          B_l = kron_{w=4..7} F(l,w)
    q' = sum_k E_k W D_k       (k = 0..3: rank-4 Schmidt split of the one
                                row/col-crossing CNOT pair; E_k, D_k are
                                constants that also absorb the row-local and
                                col-local CNOT+noise blocks)
F(l,w) = N1 @ blkdiag(1, Rz(t_z) Ry(t_y)) is the per-wire rotation+noise PTM.
Output: out[b] = P_r(b)^T q P_c(b) with P_r/P_c kron products of per-wire
encoding vectors N1 @ (1, sin x, 0, cos x).

Layer 5 starts from the constant one-hot q0 (q0[192,0]=1), so
W(5) = B^T q0^T A ... = brow^T (x) arow with arow = A(5)[192,:],
brow = B(5)[0,:] -- built as two K=1 outer-product matmuls, skipping the
dense rotation matmuls, the At/Bt tile build, and the q0 DMA entirely.

The chain runs in bf16 (PSUM accumulation stays fp32); measured accuracy vs
the fp64 reference is ~7e-3 (gate: 2e-2). Per-layer A/B tile construction is
emitted at the top of the previous layer so the DVE builds it under the
tensor engine's matmuls; PSUM->SBUF copies alternate Scalar/Vector; the
activation table and all small constants are front-loaded into the first two
DMA slots so the trig -> F-matrix -> chain critical path starts early.

Each of the 8 cores runs the identical chain and handles 4 of the 32 samples.
"""

import os
import sys

import numpy as np

sys.path.insert(0, "/opt/trn_rl_repo")

import concourse.bass as bass  # noqa: E402
import concourse.bacc as bacc  # noqa: E402
import concourse.tile as tile  # noqa: E402
from concourse import mybir  # noqa: E402

F32 = mybir.dt.float32
F32R = mybir.dt.float32r
BF16 = mybir.dt.bfloat16
AF = mybir.ActivationFunctionType

N_QUBITS = 8
DEPTH = 6
BATCH = 32
N_CORES = 8
B_PER = BATCH // N_CORES  # 4
G1, G2 = 0.0003, 0.0065

HALF_PI = float(np.pi / 2)
N_WARMUP = int(os.environ.get("QK_WARMUP", "9"))

# ---------------------------------------------------------------------------
# Constant precompute (numpy, float64 -> float32)
# ---------------------------------------------------------------------------


def _consts():
    I2 = np.eye(2, dtype=complex)
    X = np.array([[0, 1], [1, 0]], dtype=complex)
    Y = np.array([[0, -1j], [1j, 0]], dtype=complex)
    Z = np.diag([1.0, -1.0]).astype(complex)
    PAULI = [I2, X, Y, Z]

    def amp_k(g):
        return [np.array([[1, 0], [0, np.sqrt(1 - g)]], complex),
                np.array([[0, np.sqrt(g)], [0, 0]], complex)]

    def phase_k(g):
        return [np.array([[1, 0], [0, np.sqrt(1 - g)]], complex),
                np.array([[0, 0], [0, np.sqrt(g)]], complex)]

    def depol_k(p):
        s0, s = np.sqrt(1 - p), np.sqrt(p / 3.0)
        return [s0 * I2, s * X, s * Y, s * Z]

    def super_1q(kraus):
        S = np.zeros((4, 4))
        for a in range(4):
            for b in range(4):
                acc = 0j
                for K in kraus:
                    acc += np.trace(PAULI[a] @ K @ PAULI[b] @ K.conj().T)
                S[a, b] = (0.5 * acc).real
        return S

    def chan(chs):
        S = np.eye(4)
        for k in chs:
            S = super_1q(k) @ S
        return S

    N1 = chan([amp_k(G1 * 0.3), phase_k(G1 * 0.2), depol_k(G1 * 0.5)])
    N2 = chan([amp_k(G2 * 0.3), phase_k(G2 * 0.2), depol_k(G2 * 0.5)])

    CNOT = np.array(
        [[1, 0, 0, 0], [0, 1, 0, 0], [0, 0, 0, 1], [0, 0, 1, 0]], complex)
    S_CNOT = np.zeros((16, 16))
    for a1 in range(4):
        for a2 in range(4):
            PA = np.kron(PAULI[a1], PAULI[a2])
            for b1 in range(4):
                for b2 in range(4):
                    PB = np.kron(PAULI[b1], PAULI[b2])
                    S_CNOT[4 * a1 + a2, 4 * b1 + b2] = (
                        0.25 * np.trace(PA @ CNOT @ PB @ CNOT.conj().T)).real
    C2 = np.kron(N2, N2) @ S_CNOT
    C2T = C2.T

    def lift(M, pos):  # on 4 base-4 digits, digit 0 most significant
        return np.kron(np.kron(np.eye(4 ** pos), M), np.eye(4 ** (2 - pos)))

    G_c = lift(C2T, 0) @ lift(C2T, 1) @ lift(C2T, 2)
    G_r = lift(C2T, 0) @ lift(C2T, 1) @ lift(C2T, 2)  # same structure

    C4 = C2T.reshape(4, 4, 4, 4)
    R = C4.transpose(0, 2, 1, 3).reshape(16, 16)
    U, s, Vt = np.linalg.svd(R)
    rank = int((s > 1e-12).sum())
    assert rank == 4, rank
    Ds = np.zeros((4, 256, 256))
    EsT = np.zeros((4, 256, 256))
    for k in range(4):
        alpha = (np.sqrt(s[k]) * U[:, k]).reshape(4, 4)
        beta = (np.sqrt(s[k]) * Vt[k, :]).reshape(4, 4)
        E_k = G_r @ np.kron(np.eye(64), alpha)
        D_k = (np.kron(beta, np.eye(64)) @ G_c).T
        Ds[k] = D_k
        EsT[k] = E_k.T

    # selectors
    S4 = np.zeros((4, 16))
    S4t = np.zeros((4, 16))
    for p in range(16):
        S4[p >> 2, p] = 1.0
        S4t[p & 3, p] = 1.0
    S16h = np.zeros((2, 16, 128))
    S16t = np.zeros((16, 128))
    for c in range(2):
        for p in range(128):
            S16h[c, (128 * c + p) >> 4, p] = 1.0
    for p in range(128):
        S16t[p & 15, p] = 1.0

    f = np.float32
    return dict(
        N1T=N1.T.astype(f), Ds=Ds.astype(f), EsT=EsT.astype(f),
        S4=S4.astype(f), S4t=S4t.astype(f),
        S16h=S16h.astype(f), S16t=S16t.astype(f),
    )


def _to_bf16(a):
    import ml_dtypes
    return np.ascontiguousarray(a.astype(ml_dtypes.bfloat16))


# ---------------------------------------------------------------------------
# Bass kernel builder
# ---------------------------------------------------------------------------


def build_nc(mm_bf16=True):
    """One NeuronCore program: inputs xp [4,8], wt [6,8,2] -> out [4,1]."""
    C = _consts()
    mmdt = BF16 if mm_bf16 else F32R

    nc = bacc.Bacc("TRN2", target_bir_lowering=False, debug=False,
                   num_devices=N_CORES)
    xp = nc.declare_dram_parameter("xp", [B_PER, N_QUBITS], F32, isOutput=False)
    wt = nc.declare_dram_parameter("wt", [DEPTH, N_QUBITS, 2], F32, isOutput=False)
    out_d = nc.declare_dram_parameter("out", [B_PER, 1], F32, isOutput=True)

    # pack all small fp32 constants into one [16, 433] tensor -> one DMA
    pk = np.zeros((16, 433), np.float32)
    pk[0:1, 0:16] = C["N1T"].reshape(1, 16)
    pk[3, 16] = 1.0  # e3 selector column
    pk[0:4, 17:33] = C["S4"]
    pk[0:4, 33:49] = C["S4t"]
    pk[:, 49:177] = C["S16h"][0]
    pk[:, 177:305] = C["S16h"][1]
    pk[:, 305:433] = C["S16t"]
    dPack = nc.inline_tensor(pk, "cPack")
    # D stacked along columns: Dst[r, 256*k + j] = D_k[r, j]
    Dst = np.ascontiguousarray(C["Ds"].transpose(1, 0, 2).reshape(256, 1024))
    # EsT row-halves stacked along columns: EstA[c][p, 256k+j] = EsT[k][128c+p, j]
    EstA = [np.ascontiguousarray(
        C["EsT"][:, 128 * c:128 * (c + 1), :].transpose(1, 0, 2).reshape(
            128, 1024)) for c in range(2)]
    cvt = _to_bf16 if mm_bf16 else (lambda a: a)
    dDst = nc.inline_tensor(cvt(Dst), "cDst")
    dEstA = [nc.inline_tensor(cvt(EstA[c]), f"cEstA{c}") for c in range(2)]

    with tile.TileContext(nc) as tc:
        with (
            tc.tile_pool(name="cpool", bufs=1) as cpool,
            tc.tile_pool(name="abpool", bufs=1) as abpool,
            tc.tile_pool(name="wpool", bufs=2) as wpool,
            tc.tile_pool(name="qpool", bufs=2) as qpool,
            tc.tile_pool(name="ppmm", bufs=6, space="PSUM") as ppmm,
            tc.tile_pool(name="ppsm", bufs=2, space="PSUM") as ppsm,
        ):
            def cdma(dram_ap, shape, tag, dt_=F32):
                t = cpool.tile(shape, dt_, tag=tag, name=tag)
                if dt_ is F32R:
                    nc.sync.dma_start(t[:], dram_ap.bitcast(dt_))
                else:
                    nc.sync.dma_start(t[:], dram_ap)
                return t

            # dummy Sin on a memset scalar: enqueues the activation-table DMA
            # ahead of the constant DMAs so the table is loaded by the time
            # the real trig runs (the first matmul is gated on it)
            tdum = cpool.tile([1, 1], F32, tag="tdum", name="tdum")
            nc.vector.memset(tdum[:], 0.0)
            nc.scalar.activation(tdum[:], tdum[:], AF.Sin)

            # HAM pre-warm: the PE clock sits at 1.2GHz until ~3.4us of
            # sustained activity. The DMA/trig ramp leaves the PE idle for
            # ~4us at kernel start; fill it with independent matmuls on a
            # scratch tile so the real chain starts at 2.4GHz.
            tscr = cpool.tile([128, 512], BF16, tag="tscr", name="tscr")
            nc.vector.memset(tscr[:], 0.0)
            if N_WARMUP:
                ps_wu = ppmm.tile([128, 512], F32, tag="mm", name="wu")
                for i in range(N_WARMUP):
                    nc.tensor.matmul(ps_wu[:], tscr[:, 0:128], tscr[:],
                                     start=True, stop=True)
                # BIR verifier wants every PSUM write read back
                nc.vector.tensor_copy(tdum[:], ps_wu[0:1, 0:1])

            # -------- angle DMA first (critical path), then constants ------
            # one [1,128] angle tile: cols 0:96 = weight angles, 96:128 = x
            th = cpool.tile([1, 128], F32, tag="th", name="th")
            nc.sync.dma_start(
                th[:, 0:96].rearrange("p (t j) -> p t j", t=2),
                wt[:].rearrange("l w t -> () t (l w)"))
            nc.sync.dma_start(th[:, 96:128],
                              xp[:].rearrange("b w -> () (b w)"))

            tPack = cdma(dPack[:, :], [16, 433], "pack")
            tN1c = tPack[0:1, 0:16]
            tE3 = tPack[0:4, 16:17]
            tS4 = tPack[0:4, 17:33]
            tS4t = tPack[0:4, 33:49]
            tS16h = [tPack[0:16, 49:177], tPack[0:16, 177:305]]
            tS16t = tPack[0:16, 305:433]
            # f32r view of the same constants (weights side of the fp32r
            # selector matmuls; DMA-sourced so walrus accepts the bitcast)
            tPackR = cdma(dPack[:, :], [16, 433], "packR", F32R)
            tN1cR = tPackR[0:1, 0:16]
            tE3R = tPackR[0:4, 16:17]
            tS4R = tPackR[0:4, 17:33]
            tS4tR = tPackR[0:4, 33:49]
            tS16hR = [tPackR[0:16, 49:177], tPackR[0:16, 177:305]]
            tS16tR = tPackR[0:16, 305:433]
            tDst = [cdma(dDst[128 * c:128 * (c + 1), :], [128, 1024],
                         f"dst{c}", mmdt) for c in range(2)]
            tEstA = [cdma(dEstA[c][:, :], [128, 1024], f"esa{c}", mmdt)
                     for c in range(2)]
            tEsT = [[tEstA[c][:, 256 * k:256 * (k + 1)] for c in range(2)]
                    for k in range(4)]

            tones = cpool.tile([128, 1], F32, tag="ones", name="ones")
            nc.vector.memset(tones[:], 1.0)
            tpi2 = cpool.tile([1, 1], F32, tag="pi2", name="pi2")
            nc.vector.memset(tpi2[:], HALF_PI)
            tone_row = cpool.tile([1, 48], F32, tag="tone_row", name="tone_row")
            nc.vector.memset(tone_row[:], 1.0)

            # ---------------- angles -> trig (weights + x together) --------
            sn = cpool.tile([1, 128], F32, tag="sn", name="sn")
            cs = cpool.tile([1, 128], F32, tag="cs", name="cs")
            nc.scalar.activation(sn[:], th[:], AF.Sin)
            nc.scalar.activation(cs[:], th[:], AF.Sin, bias=tpi2[:])
            sy, szr = sn[0:1, 0:48], sn[0:1, 48:96]
            cy, czr = cs[0:1, 0:48], cs[0:1, 48:96]
            xsin, xcos = sn[0:1, 96:128], cs[0:1, 96:128]
            # Rotblk rows as [1,192] vectors (r0..r3), then
            # F_all = sum_r N1[:, r] (x) row_r  via K=1 accumulating matmuls.
            # memsets first (no trig dep), then spread the row fills over
            # vector/scalar/gpsimd so the serial DVE chain shortens.
            # rv padded to 256 cols (zeros beyond 192) so the F-build matmuls
            # can run as fp32r with N>=256 (1 cyc/row vs fp32's 4); tiles are
            # f32r-typed so walrus sees explicitly-rounded matmul inputs
            # (memset can't write f32r directly — zero via fp32->f32r copy)
            zrow = cpool.tile([16, 256], F32, tag="zrow", name="zrow")
            nc.vector.memset(zrow[:], 0.0)
            rv = []
            for r in range(4):
                t = cpool.tile([1, 256], F32R, tag=f"rv{r}", name=f"rv{r}")
                nc.vector.tensor_copy(t[:], zrow[0:1, :])
                rv.append(t)
            rvv = [t[:, 0:192].rearrange("p (j n) -> p j n", n=4) for t in rv]

            def c3(a):
                return a.rearrange("p j -> p j ()")

            nc.vector.tensor_copy(rvv[0][:, :, 0:1], c3(tone_row[0:1, :]))

            pcc = cpool.tile([1, 48], F32, tag="pcc", name="pcc")  # cz*cy
            pcs = cpool.tile([1, 48], F32, tag="pcs", name="pcs")  # cz*sy
            psc = cpool.tile([1, 48], F32, tag="psc", name="psc")  # sz*cy
            pss = cpool.tile([1, 48], F32, tag="pss", name="pss")  # sz*sy
            nc.vector.tensor_mul(pcc[:], czr, cy)
            nc.vector.tensor_mul(pcs[:], czr, sy)
            nc.vector.tensor_mul(psc[:], szr, cy)
            nc.vector.tensor_mul(pss[:], szr, sy)

            nc.vector.tensor_copy(rvv[2][:, :, 2:3], c3(czr))
            nc.vector.tensor_copy(rvv[3][:, :, 3:4], c3(cy))
            nc.vector.tensor_copy(rvv[1][:, :, 1:2], c3(pcc[:]))
            nc.scalar.mul(rvv[1][:, :, 2:3], c3(szr), -1.0)
            nc.vector.tensor_copy(rvv[1][:, :, 3:4], c3(pcs[:]))
            nc.vector.tensor_copy(rvv[2][:, :, 1:2], c3(psc[:]))
            nc.vector.tensor_copy(rvv[2][:, :, 3:4], c3(pss[:]))
            nc.scalar.mul(rvv[3][:, :, 1:2], c3(sy), -1.0)

            ps_f = ppsm.tile([4, 256], F32, tag="sm", name="ps_f")
            for r in range(4):
                nc.tensor.matmul(ps_f[:], tN1cR[0:1, 4 * r:4 * (r + 1)],
                                 rv[r][:], start=(r == 0), stop=(r == 3))
            fall = cpool.tile([4, 256], F32R, tag="fall", name="fall")
            nc.vector.tensor_copy(fall[:], ps_f[:])

            # -------- layer-5 rank-1 start: arow = A(5)[192,:], brow = B(5)[0,:]
            # fall[m, 4*(8l+w)+n] = F(l,w)[m,n]; l=5 -> cols 160+4w..164+4w
            # DVE can't read from partition 3, so extract fall row 3 to
            # partition 0 with a K=4 selector matmul first.
            ps_f3 = ppsm.tile([1, 256], F32, tag="sm", name="ps_f3")
            nc.tensor.matmul(ps_f3[:], tE3R[:], fall[:], start=True, stop=True)
            fall3 = cpool.tile([1, 192], F32, tag="fall3", name="fall3")
            nc.vector.tensor_copy(fall3[:], ps_f3[:, 0:192])

            def fvec(w, row):
                o = 4 * (8 * 5 + w)
                if row == 3:
                    return fall3[0:1, o:o + 4]
                return fall[0:1, o:o + 4].bitcast(F32)

            def kron2(dst16, va, vb):
                nc.vector.tensor_mul(
                    dst16[:].rearrange("p (a b) -> p a b", a=4),
                    va.unsqueeze(2).broadcast_to([1, 4, 4]),
                    vb.unsqueeze(1).broadcast_to([1, 4, 4]))

            v01 = cpool.tile([1, 16], F32, tag="v01", name="v01")
            v23 = cpool.tile([1, 16], F32, tag="v23", name="v23")
            v45 = cpool.tile([1, 16], F32, tag="v45", name="v45")
            v67 = cpool.tile([1, 16], F32, tag="v67", name="v67")
            kron2(v01, fvec(0, 3), fvec(1, 0))
            kron2(v23, fvec(2, 0), fvec(3, 0))
            kron2(v45, fvec(4, 0), fvec(5, 0))
            kron2(v67, fvec(6, 0), fvec(7, 0))
            arow = cpool.tile([1, 256], mmdt, tag="arow", name="arow")
            brow = cpool.tile([1, 256], mmdt, tag="brow", name="brow")
            nc.vector.tensor_mul(
                arow[:].rearrange("p (a b) -> p a b", a=16),
                v01[:].unsqueeze(2).broadcast_to([1, 16, 16]),
                v23[:].unsqueeze(1).broadcast_to([1, 16, 16]))
            nc.vector.tensor_mul(
                brow[:].rearrange("p (a b) -> p a b", a=16),
                v45[:].unsqueeze(2).broadcast_to([1, 16, 16]),
                v67[:].unsqueeze(1).broadcast_to([1, 16, 16]))

            # ------------- batched selector expansions -------------
            # t1a[p, 4j+n] = F_j[p>>2, n]; t2a[p, 4j+n] = F_j[p&3, n]
            ps1 = ppsm.tile([16, 256], F32, tag="sm", name="ps1")
            nc.tensor.matmul(ps1[:], tS4R[:], fall[:], start=True, stop=True)
            t1a = cpool.tile([16, 192], F32, tag="t1a", name="t1a")
            nc.scalar.copy(t1a[:], ps1[:, 0:192])
            ps2 = ppsm.tile([16, 256], F32, tag="sm", name="ps2")
            nc.tensor.matmul(ps2[:], tS4tR[:], fall[:], start=True, stop=True)
            t2a = cpool.tile([16, 192], F32, tag="t2a", name="t2a")
            nc.scalar.copy(t2a[:], ps2[:, 0:192])

            # pair-kron tiles for all layers: fpa[pos][p, 16l + 4a+b]
            # (built on GpSimd to keep the DVE free for PSUM copies)
            fpa = []
            for pos in range(4):
                fp = abpool.tile([16, 256], F32R, tag=f"fpa{pos}",
                                 name=f"fpa{pos}")
                nc.vector.tensor_copy(fp[:, 96:256], zrow[:, 96:256])
                for l in range(DEPTH):
                    o = 32 * l + 8 * pos
                    nc.vector.tensor_mul(
                        fp[:, 16 * l:16 * (l + 1)].rearrange(
                            "p (a b) -> p a b", a=4),
                        t1a[:, o:o + 4].unsqueeze(2).broadcast_to([16, 4, 4]),
                        t2a[:, o + 4:o + 8].unsqueeze(1).broadcast_to([16, 4, 4]),
                    )
                fpa.append(fp)

            # quad selector expansions, batched over layers: [128, 96]
            # (fp32r with N=256 padding: 1 cyc/row on the PE vs fp32's 4)
            def sel_expand(sel, fp_all, tag):
                ps = ppsm.tile([128, 256], F32, tag="sm", name=f"ps{tag}")
                nc.tensor.matmul(ps[:], sel[:], fp_all[:],
                                 start=True, stop=True)
                t = cpool.tile([128, 96], F32, tag=tag, name=tag)
                nc.scalar.copy(t[:], ps[:, 0:96])
                return t

            zA = [sel_expand(tS16hR[c], fpa[0], f"zA{c}") for c in range(2)]
            yA = sel_expand(tS16tR, fpa[1], "yA")
            zB = [sel_expand(tS16hR[c], fpa[2], f"zB{c}") for c in range(2)]
            yB = sel_expand(tS16tR, fpa[3], "yB")

            # per-layer A/B kron tile build (GpSimd, SBUF->SBUF), called
            # lazily mid-chain; keeps DVE/ACT free for PSUM copies
            def build_ab(l):
                sl = slice(16 * l, 16 * (l + 1))
                Al, Bl = [], []
                for c in range(2):
                    ab = abpool.tile([128, 256], mmdt, tag=f"A{l}_{c}",
                                     name=f"A{l}_{c}")
                    nc.vector.tensor_mul(
                        ab[:].rearrange("p (a b) -> p a b", a=16),
                        zA[c][:, sl].unsqueeze(2).broadcast_to([128, 16, 16]),
                        yA[:, sl].unsqueeze(1).broadcast_to([128, 16, 16]),
                    )
                    Al.append(ab)
                    bb = abpool.tile([128, 256], mmdt, tag=f"B{l}_{c}",
                                     name=f"B{l}_{c}")
                    nc.vector.tensor_mul(
                        bb[:].rearrange("p (a b) -> p a b", a=16),
                        zB[c][:, sl].unsqueeze(2).broadcast_to([128, 16, 16]),
                        yB[:, sl].unsqueeze(1).broadcast_to([128, 16, 16]),
                    )
                    Bl.append(bb)
                return Al, Bl

            # ---------------- the chain ----------------
            # PSUM->SBUF copies alternate Scalar/Vector per psum tile (the
            # two m-halves live in different banks, so the engines overlap).
            copy_engines = [nc.scalar.copy,
                            nc.vector.tensor_copy]
            copy_flip = [0]

            def copy_piece(dst, src):
                copy_engines[copy_flip[0] % len(copy_engines)](dst, src)
                copy_flip[0] += 1

            # ---- low-rank phase: the pullback has rank 1 at the layer-5
            # start and rank 4^s after s layers, so layers 5/4/3 run on
            # rank-4/16/64 factors (q = L R^T) with tiny-N matmuls instead
            # of dense 256x256 sandwiches. All stationary operands are the
            # same At/Bt/tEstA/tDst tiles the dense chain uses.
            #
            # seed: W(5) = a b^T (a = arow^T, b = brow^T) =>
            #   L0 = [E_k a], R0 = [D_k^T b]  (rank 4)
            onebf = cpool.tile([1, 1], mmdt, tag="onebf", name="onebf")
            nc.vector.memset(onebf[:], 1.0)
            pcol = ppsm.tile([128, 4], F32, tag="sm", name="pcol")
            for ci, srow in enumerate((arow, brow)):
                for c in range(2):
                    nc.tensor.matmul(pcol[:, 2 * ci + c:2 * ci + c + 1],
                                     srow[0:1, 128 * c:128 * (c + 1)],
                                     onebf[:], start=True, stop=True,
                                     skip_group_check=True)
            abcol = cpool.tile([128, 4], mmdt, tag="abcol", name="abcol")
            nc.vector.tensor_copy(abcol[:], pcol[:])
            acol = [abcol[:, 0:1], abcol[:, 1:2]]
            bcol = [abcol[:, 2:3], abcol[:, 3:4]]

            # A/B tiles for the factored layers (emitted early: DVE builds
            # them under the seed's PE work)
            At4, Bt4 = build_ab(4)
            At3, Bt3 = build_ab(3)

            L0, R0 = [], []
            for m in range(2):
                psL = ppmm.tile([128, 4], F32, tag="mm", name=f"psL0{m}")
                psR = ppmm.tile([128, 4], F32, tag="mm", name=f"psR0{m}")
                for k in range(4):
                    for c in range(2):
                        nc.tensor.matmul(
                            psL[:, k:k + 1],
                            tEstA[c][:, 256 * k + 128 * m:
                                     256 * k + 128 * (m + 1)],
                            acol[c], start=(c == 0), stop=(c == 1),
                            skip_group_check=True)
                        nc.tensor.matmul(
                            psR[:, k:k + 1],
                            tDst[c][:, 256 * k + 128 * m:
                                    256 * k + 128 * (m + 1)],
                            bcol[c], start=(c == 0), stop=(c == 1),
                            skip_group_check=True)
                tL = wpool.tile([128, 4], mmdt, tag=f"L0{m}", name=f"L0{m}")
                tR = wpool.tile([128, 4], mmdt, tag=f"R0{m}", name=f"R0{m}")
                nc.scalar.copy(tL[:], psL[:])
                nc.vector.tensor_copy(tR[:], psR[:])
                L0.append(tL)
                R0.append(tR)

            # one factored layer: (L, R, r) -> ([E_k A L], [D_k^T B^T R])
            def factored_layer(Lf, Rf, Atl, Btl, r, tagp):
                XL, XR = [], []
                for m in range(2):
                    psX = ppmm.tile([128, r], F32, tag="mm", name=f"psXL{m}")
                    psY = ppmm.tile([128, r], F32, tag="mm", name=f"psXR{m}")
                    for c in range(2):
                        nc.tensor.matmul(
                            psX[:], Atl[c][:, 128 * m:128 * (m + 1)],
                            Lf[c][:], start=(c == 0), stop=(c == 1))
                        nc.tensor.matmul(
                            psY[:], Btl[c][:, 128 * m:128 * (m + 1)],
                            Rf[c][:], start=(c == 0), stop=(c == 1))
                    tX = wpool.tile([128, r], mmdt, tag=f"XL{tagp}{m}",
                                    name=f"XL{tagp}{m}")
                    tY = wpool.tile([128, r], mmdt, tag=f"XR{tagp}{m}",
                                    name=f"XR{tagp}{m}")
                    nc.scalar.copy(tX[:], psX[:])
                    nc.vector.tensor_copy(tY[:], psY[:])
                    XL.append(tX)
                    XR.append(tY)
                Lout, Rout = [], []
                for m in range(2):
                    psL = ppmm.tile([128, 4 * r], F32, tag="mm",
                                    name=f"psL{tagp}{m}")
                    psR = ppmm.tile([128, 4 * r], F32, tag="mm",
                                    name=f"psR{tagp}{m}")
                    for k in range(4):
                        for c in range(2):
                            nc.tensor.matmul(
                                psL[:, r * k:r * (k + 1)],
                                tEstA[c][:, 256 * k + 128 * m:
                                         256 * k + 128 * (m + 1)],
                                XL[c][:], start=(c == 0), stop=(c == 1),
                                skip_group_check=True)
                            nc.tensor.matmul(
                                psR[:, r * k:r * (k + 1)],
                                tDst[c][:, 256 * k + 128 * m:
                                        256 * k + 128 * (m + 1)],
                                XR[c][:], start=(c == 0), stop=(c == 1),
                                skip_group_check=True)
                    tL = wpool.tile([128, 4 * r], mmdt, tag=f"Lo{tagp}{m}",
                                    name=f"Lo{tagp}{m}")
                    tR = wpool.tile([128, 4 * r], mmdt, tag=f"Ro{tagp}{m}",
                                    name=f"Ro{tagp}{m}")
                    nc.scalar.copy(tL[:], psL[:])
                    nc.vector.tensor_copy(tR[:], psR[:])
                    Lout.append(tL)
                    Rout.append(tR)
                return Lout, Rout

            # layer 4 on rank-4 factors -> rank 16
            L1, R1 = factored_layer(L0, R0, At4, Bt4, 4, "f1")

            # layer 3: rotate (A L1 / B^T R1 at r=16), then produce the
            # E/D-expanded factors TRANSPOSED ([64, 256], k-blocks at
            # partition 32k) so the rank-64 q3 materializes directly.
            At2, Bt2 = build_ab(2)
            XL2, XR2 = [], []
            for m in range(2):
                psX = ppmm.tile([128, 16], F32, tag="mm", name=f"psXL2{m}")
                psY = ppmm.tile([128, 16], F32, tag="mm", name=f"psXR2{m}")
                for c in range(2):
                    nc.tensor.matmul(psX[:], At3[c][:, 128 * m:128 * (m + 1)],
                                     L1[c][:], start=(c == 0), stop=(c == 1))
                    nc.tensor.matmul(psY[:], Bt3[c][:, 128 * m:128 * (m + 1)],
                                     R1[c][:], start=(c == 0), stop=(c == 1))
                tX = wpool.tile([128, 16], mmdt, tag=f"XL2{m}",
                                name=f"XL2{m}")
                tY = wpool.tile([128, 16], mmdt, tag=f"XR2{m}",
                                name=f"XR2{m}")
                nc.scalar.copy(tX[:], psX[:])
                nc.vector.tensor_copy(tY[:], psY[:])
                XL2.append(tX)
                XR2.append(tY)
            # (partition offsets are limited to {0, 32, 64} and quadrant 3
            # is unusable, so the 4 k-blocks go 2-per-tile at offsets 0/32)
            LTs, RTs = [], []
            for h in range(2):
                psLT = ppmm.tile([64, 256], F32, tag="mm", name=f"psLT{h}")
                psRT = ppmm.tile([64, 256], F32, tag="mm", name=f"psRT{h}")
                for kk in range(2):
                    k = 2 * h + kk
                    for c in range(2):
                        nc.tensor.matmul(
                            psLT[32 * kk:32 * kk + 16, :],
                            XL2[c][:], tEstA[c][:, 256 * k:256 * (k + 1)],
                            start=(c == 0), stop=(c == 1),
                            skip_group_check=True)
                        nc.tensor.matmul(
                            psRT[32 * kk:32 * kk + 16, :],
                            XR2[c][:], tDst[c][:, 256 * k:256 * (k + 1)],
                            start=(c == 0), stop=(c == 1),
                            skip_group_check=True)
                LT = wpool.tile([64, 256], mmdt, tag=f"LT{h}", name=f"LT{h}")
                RT = wpool.tile([64, 256], mmdt, tag=f"RT{h}", name=f"RT{h}")
                for kk in range(2):
                    copy_piece(LT[32 * kk:32 * kk + 16, :],
                               psLT[32 * kk:32 * kk + 16, :])
                    copy_piece(RT[32 * kk:32 * kk + 16, :],
                               psRT[32 * kk:32 * kk + 16, :])
                LTs.append(LT)
                RTs.append(RT)

            # materialize q3 = sum_k (E_k A L1')(D_k^T B^T R1')^T
            q_sb = []
            for m in range(2):
                ps = ppmm.tile([128, 256], F32, tag="mm", name=f"ps_q3{m}")
                for h in range(2):
                    for kk in range(2):
                        nc.tensor.matmul(
                            ps[:], LTs[h][32 * kk:32 * kk + 16,
                                          128 * m:128 * (m + 1)],
                            RTs[h][32 * kk:32 * kk + 16, :],
                            start=(h == 0 and kk == 0),
                            stop=(h == 1 and kk == 1))
                t = qpool.tile([128, 256], mmdt, tag=f"q{m}", name=f"q{m}")
                copy_piece(t[:], ps[:])
                q_sb.append(t)

            At, Bt = At2, Bt2
            At_next = Bt_next = None
            for s in range(3, DEPTH):
                l = DEPTH - 1 - s
                # emit next layer's A/B tile build first so the DVE runs it
                # under this layer's matmuls (no dep on the chain)
                if l > 0:
                    At_next, Bt_next = build_ab(l - 1)
                if True:
                    # Tp = q^T @ A   [C, R']
                    tp_sb = []
                    for m in range(2):
                        ps = ppmm.tile([128, 256], F32, tag="mm", name="ps_tp")
                        for c in range(2):
                            nc.tensor.matmul(
                                ps[:], q_sb[c][:, 128 * m:128 * (m + 1)],
                                At[c][:], start=(c == 0), stop=(c == 1))
                        t = wpool.tile([128, 256], mmdt, tag=f"tp{m}",
                                       name=f"tp{m}")
                        copy_piece(t[:], ps[:])
                        tp_sb.append(t)
                    # Wp = B^T @ Tp  [C', R']
                    wp_sb = []
                    for m in range(2):
                        ps = ppmm.tile([128, 256], F32, tag="mm", name="ps_wp")
                        for c in range(2):
                            nc.tensor.matmul(
                                ps[:], Bt[c][:, 128 * m:128 * (m + 1)],
                                tp_sb[c][:], start=(c == 0), stop=(c == 1))
                        t = wpool.tile([128, 256], mmdt, tag=f"wp{m}",
                                       name=f"wp{m}")
                        copy_piece(t[:], ps[:])
                        wp_sb.append(t)
                # U = W @ [D_0|D_1|D_2|D_3]   [R', (k,j)] as [128, 1024] tiles
                uall = []
                for m in range(2):
                    u = wpool.tile([128, 1024], mmdt, tag=f"u{m}", name=f"u{m}")
                    for nh in range(2):
                        ps = ppmm.tile([128, 512], F32, tag="mm", name="ps_u")
                        for c in range(2):
                            nc.tensor.matmul(
                                ps[:], wp_sb[c][:, 128 * m:128 * (m + 1)],
                                tDst[c][:, 512 * nh:512 * (nh + 1)],
                                start=(c == 0), stop=(c == 1))
                        copy_piece(u[:, 512 * nh:512 * (nh + 1)], ps[:])
                    uall.append(u)
                # q' = sum_k E_k U_k
                q_new = []
                for m in range(2):
                    ps = ppmm.tile([128, 256], F32, tag="mm", name="ps_q")
                    first = True
                    for c in range(2):
                        for k in range(4):
                            nc.tensor.matmul(
                                ps[:], tEsT[k][c][:, 128 * m:128 * (m + 1)],
                                uall[c][:, 256 * k:256 * (k + 1)],
                                start=first, stop=(c == 1 and k == 3))
                            first = False
                    t = qpool.tile([128, 256], mmdt, tag=f"q{m}", name=f"q{m}")
                    copy_piece(t[:], ps[:])
                    q_new.append(t)
                q_sb = q_new
                At, Bt = At_next, Bt_next
                if l == 2:
                    # ---------------- encoding vectors (for the finale) ----
                    ones32 = cpool.tile([1, 32], F32, tag="ones32",
                                        name="ones32")
                    nc.vector.memset(ones32[:], 1.0)
                    ps_e = ppsm.tile([4, 32], F32, tag="sm", name="ps_e")
                    for i, (r, src_row) in enumerate(
                            [(0, ones32[:]), (1, xsin), (3, xcos)]):
                        nc.tensor.matmul(ps_e[:],
                                         tN1c[0:1, 4 * r:4 * (r + 1)],
                                         src_row, start=(i == 0), stop=(i == 2))
                    aenc = cpool.tile([4, 32], F32, tag="aenc", name="aenc")
                    nc.vector.tensor_copy(aenc[:], ps_e[:])

                    pse1 = ppsm.tile([16, 32], F32, tag="sm", name="pse1")
                    nc.tensor.matmul(pse1[:], tS4[:], aenc[:],
                                     start=True, stop=True)
                    s1e = cpool.tile([16, 32], F32, tag="s1e", name="s1e")
                    nc.vector.tensor_copy(s1e[:], pse1[:])
                    pse2 = ppsm.tile([16, 32], F32, tag="sm", name="pse2")
                    nc.tensor.matmul(pse2[:], tS4t[:], aenc[:],
                                     start=True, stop=True)
                    s2e = cpool.tile([16, 32], F32, tag="s2e", name="s2e")
                    nc.vector.tensor_copy(s2e[:], pse2[:])

                    def wcol(t, w):
                        return t[:].rearrange("p (b w) -> p b w", w=8)[:, :, w]

                    # ahi = [a01 | a45], alo = [a23 | a67] (cols = 4 samples)
                    ahi = cpool.tile([16, 8], F32, tag="ahi", name="ahi")
                    alo = cpool.tile([16, 8], F32, tag="alo", name="alo")
                    nc.vector.tensor_mul(ahi[:, 0:4], wcol(s1e, 0), wcol(s2e, 1))
                    nc.vector.tensor_mul(ahi[:, 4:8], wcol(s1e, 4), wcol(s2e, 5))
                    nc.vector.tensor_mul(alo[:, 0:4], wcol(s1e, 2), wcol(s2e, 3))
                    nc.vector.tensor_mul(alo[:, 4:8], wcol(s1e, 6), wcol(s2e, 7))

                    psy = ppsm.tile([128, 8], F32, tag="sm", name="psy")
                    nc.tensor.matmul(psy[:], tS16t[:], alo[:],
                                     start=True, stop=True)
                    yq = cpool.tile([128, 8], F32, tag="yq", name="yq")
                    nc.vector.tensor_copy(yq[:], psy[:])
                    Pr = []
                    Pc = []
                    for c in range(2):
                        psz = ppsm.tile([128, 8], F32, tag="sm", name="psz")
                        nc.tensor.matmul(psz[:], tS16h[c][:], ahi[:],
                                         start=True, stop=True)
                        pr = cpool.tile([128, B_PER], mmdt, tag=f"pr{c}",
                                        name=f"pr{c}")
                        nc.vector.tensor_mul(pr[:], psz[:, 0:4], yq[:, 0:4])
                        pc = cpool.tile([128, B_PER], F32, tag=f"pc{c}",
                                        name=f"pc{c}")
                        nc.vector.tensor_mul(pc[:], psz[:, 4:8], yq[:, 4:8])
                        Pr.append(pr)
                        Pc.append(pc)

            # ---------------- final contraction ----------------
            h_sb = []
            for m in range(2):
                ps = ppsm.tile([128, B_PER], F32, tag="sm", name="ps_g")
                for c in range(2):
                    nc.tensor.matmul(
                        ps[:], q_sb[c][:, 128 * m:128 * (m + 1)],
                        Pr[c][:], start=(c == 0), stop=(c == 1))
                h = cpool.tile([128, B_PER], F32, tag=f"h{m}", name=f"h{m}")
                nc.vector.tensor_mul(h[:], ps[:], Pc[m][:])
                h_sb.append(h)
            ps_o = ppsm.tile([B_PER, 1], F32, tag="sm", name="ps_o")
            for m in range(2):
                nc.tensor.matmul(ps_o[:], h_sb[m][:], tones[:],
                                 start=(m == 0), stop=(m == 1))
            out_sb = cpool.tile([B_PER, 1], F32, tag="osb", name="osb")
            nc.vector.tensor_copy(out_sb[:], ps_o[:])
            nc.sync.dma_start(out_d[:, :], out_sb[:])

    nc.compile()
    return nc


# ---------------------------------------------------------------------------
# Host entry point
# ---------------------------------------------------------------------------

_NC = None


def _get_nc():
    global _NC
    if _NC is None:
        _NC = build_nc(mm_bf16=os.environ.get("QK_MM_F32R") != "1")
    return _NC


def _maybe_enable_ldw_opt():
    if os.environ.get("QK_LDW_OPT") != "1":
        return
    from concourse.compiler_utils import get_compiler_flags, set_compiler_flags

    flags = [f.replace("--enable-ldw-opt=false", "--enable-ldw-opt=true")
             for f in get_compiler_flags()]
    set_compiler_flags(flags)


def kernel(x: np.ndarray, weights: np.ndarray) -> np.ndarray:
    from concourse.bass_utils import run_bass_kernel_spmd

    _maybe_enable_ldw_opt()

    nc = _get_nc()
    x = np.ascontiguousarray(x, dtype=np.float32)
    weights = np.ascontiguousarray(weights, dtype=np.float32)
    in_maps = [
        {"xp": x[i * B_PER:(i + 1) * B_PER], "wt": weights}
        for i in range(N_CORES)
    ]
    res = run_bass_kernel_spmd(nc, in_maps, list(range(N_CORES)))
    out = np.concatenate([res.results[i]["out"] for i in range(N_CORES)], axis=0)
    return out.astype(np.float32)



# revision 35
# speedup vs baseline: 1.1811x; 1.1811x over previous
"""Trainium2 Bass kernel for ConfigurableNoisyQuantumLayer.

Math: the circuit is a fixed sequence of single-qubit rotations, CNOTs and
noise channels acting on an 8-qubit density matrix, batched over 32 inputs x.
In the (real) Pauli-transfer-matrix picture every channel is a real 4^n x 4^n
matrix. We pull the observable Z_0 back through the 6 layers (Heisenberg
picture) -- one shared real (4^4)x(4^4)=256x256 matrix chain independent of
the batch -- then contract with per-sample product-state Pauli vectors.

Per adjoint layer (l = 5..0), with q the 256x256 pullback matrix
(rows = wires 0-3 pair-index, cols = wires 4-7):
    T  = A_l q          A_l = kron_{w=0..3} F(l,w)^T
    W  = T B_l          B_l = kron_{w=4..7} F(l,w)
    q' = sum_k E_k W D_k       (k = 0..3: rank-4 Schmidt split of the one
                                row/col-crossing CNOT pair; E_k, D_k are
                                constants that also absorb the row-local and
                                col-local CNOT+noise blocks)
F(l,w) = N1 @ blkdiag(1, Rz(t_z) Ry(t_y)) is the per-wire rotation+noise PTM.
Output: out[b] = P_r(b)^T q P_c(b) with P_r/P_c kron products of per-wire
encoding vectors N1 @ (1, sin x, 0, cos x).

Layer 5 starts from the constant one-hot q0 (q0[192,0]=1), so
W(5) = B^T q0^T A ... = brow^T (x) arow with arow = A(5)[192,:],
brow = B(5)[0,:] -- built as two K=1 outer-product matmuls, skipping the
dense rotation matmuls, the At/Bt tile build, and the q0 DMA entirely.

The chain runs in bf16 (PSUM accumulation stays fp32); measured accuracy vs
the fp64 reference is ~7e-3 (gate: 2e-2). Per-layer A/B tile construction is
emitted at the top of the previous layer so the DVE builds it under the
tensor engine's matmuls; PSUM->SBUF copies alternate Scalar/Vector; the
activation table and all small constants are front-loaded into the first two
DMA slots so the trig -> F-matrix -> chain critical path starts early.

Each of the 8 cores runs the identical chain and handles 4 of the 32 samples.
"""

import os
import sys

import numpy as np

sys.path.insert(0, "/opt/trn_rl_repo")

import concourse.bass as bass  # noqa: E402
import concourse.bacc as bacc  # noqa: E402
import concourse.tile as tile  # noqa: E402
from concourse import mybir  # noqa: E402

F32 = mybir.dt.float32
F32R = mybir.dt.float32r
BF16 = mybir.dt.bfloat16
AF = mybir.ActivationFunctionType

N_QUBITS = 8
DEPTH = 6
BATCH = 32
N_CORES = 8
B_PER = BATCH // N_CORES  # 4
G1, G2 = 0.0003, 0.0065

HALF_PI = float(np.pi / 2)
N_WARMUP = int(os.environ.get("QK_WARMUP", "9"))

# ---------------------------------------------------------------------------
# Constant precompute (numpy, float64 -> float32)
# ---------------------------------------------------------------------------


def _consts():
    I2 = np.eye(2, dtype=complex)
    X = np.array([[0, 1], [1, 0]], dtype=complex)
    Y = np.array([[0, -1j], [1j, 0]], dtype=complex)
    Z = np.diag([1.0, -1.0]).astype(complex)
    PAULI = [I2, X, Y, Z]

    def amp_k(g):
        return [np.array([[1, 0], [0, np.sqrt(1 - g)]], complex),
                np.array([[0, np.sqrt(g)], [0, 0]], complex)]

    def phase_k(g):
        return [np.array([[1, 0], [0, np.sqrt(1 - g)]], complex),
                np.array([[0, 0], [0, np.sqrt(g)]], complex)]

    def depol_k(p):
        s0, s = np.sqrt(1 - p), np.sqrt(p / 3.0)
        return [s0 * I2, s * X, s * Y, s * Z]

    def super_1q(kraus):
        S = np.zeros((4, 4))
        for a in range(4):
            for b in range(4):
                acc = 0j
                for K in kraus:
                    acc += np.trace(PAULI[a] @ K @ PAULI[b] @ K.conj().T)
                S[a, b] = (0.5 * acc).real
        return S

    def chan(chs):
        S = np.eye(4)
        for k in chs:
            S = super_1q(k) @ S
        return S

    N1 = chan([amp_k(G1 * 0.3), phase_k(G1 * 0.2), depol_k(G1 * 0.5)])
    N2 = chan([amp_k(G2 * 0.3), phase_k(G2 * 0.2), depol_k(G2 * 0.5)])

    CNOT = np.array(
        [[1, 0, 0, 0], [0, 1, 0, 0], [0, 0, 0, 1], [0, 0, 1, 0]], complex)
    S_CNOT = np.zeros((16, 16))
    for a1 in range(4):
        for a2 in range(4):
            PA = np.kron(PAULI[a1], PAULI[a2])
            for b1 in range(4):
                for b2 in range(4):
                    PB = np.kron(PAULI[b1], PAULI[b2])
                    S_CNOT[4 * a1 + a2, 4 * b1 + b2] = (
                        0.25 * np.trace(PA @ CNOT @ PB @ CNOT.conj().T)).real
    C2 = np.kron(N2, N2) @ S_CNOT
    C2T = C2.T

    def lift(M, pos):  # on 4 base-4 digits, digit 0 most significant
        return np.kron(np.kron(np.eye(4 ** pos), M), np.eye(4 ** (2 - pos)))

    G_c = lift(C2T, 0) @ lift(C2T, 1) @ lift(C2T, 2)
    G_r = lift(C2T, 0) @ lift(C2T, 1) @ lift(C2T, 2)  # same structure

    C4 = C2T.reshape(4, 4, 4, 4)
    R = C4.transpose(0, 2, 1, 3).reshape(16, 16)
    U, s, Vt = np.linalg.svd(R)
    rank = int((s > 1e-12).sum())
    assert rank == 4, rank
    Ds = np.zeros((4, 256, 256))
    EsT = np.zeros((4, 256, 256))
    for k in range(4):
        alpha = (np.sqrt(s[k]) * U[:, k]).reshape(4, 4)
        beta = (np.sqrt(s[k]) * Vt[k, :]).reshape(4, 4)
        E_k = G_r @ np.kron(np.eye(64), alpha)
        D_k = (np.kron(beta, np.eye(64)) @ G_c).T
        Ds[k] = D_k
        EsT[k] = E_k.T

    # selectors
    S4 = np.zeros((4, 16))
    S4t = np.zeros((4, 16))
    for p in range(16):
        S4[p >> 2, p] = 1.0
        S4t[p & 3, p] = 1.0
    S16h = np.zeros((2, 16, 128))
    S16t = np.zeros((16, 128))
    for c in range(2):
        for p in range(128):
            S16h[c, (128 * c + p) >> 4, p] = 1.0
    for p in range(128):
        S16t[p & 15, p] = 1.0

    f = np.float32
    return dict(
        N1T=N1.T.astype(f), Ds=Ds.astype(f), EsT=EsT.astype(f),
        S4=S4.astype(f), S4t=S4t.astype(f),
        S16h=S16h.astype(f), S16t=S16t.astype(f),
    )


def _to_bf16(a):
    import ml_dtypes
    return np.ascontiguousarray(a.astype(ml_dtypes.bfloat16))


# ---------------------------------------------------------------------------
# Bass kernel builder
# ---------------------------------------------------------------------------


def build_nc(mm_bf16=True):
    """One NeuronCore program: inputs xp [4,8], wt [6,8,2] -> out [4,1]."""
    C = _consts()
    mmdt = BF16 if mm_bf16 else F32R

    nc = bacc.Bacc("TRN2", target_bir_lowering=False, debug=False,
                   num_devices=N_CORES)
    xp = nc.declare_dram_parameter("xp", [B_PER, N_QUBITS], F32, isOutput=False)
    wt = nc.declare_dram_parameter("wt", [DEPTH, N_QUBITS, 2], F32, isOutput=False)
    out_d = nc.declare_dram_parameter("out", [B_PER, 1], F32, isOutput=True)

    # pack all small fp32 constants into one [16, 433] tensor -> one DMA
    pk = np.zeros((16, 433), np.float32)
    pk[0:1, 0:16] = C["N1T"].reshape(1, 16)
    pk[3, 16] = 1.0  # e3 selector column
    pk[0:4, 17:33] = C["S4"]
    pk[0:4, 33:49] = C["S4t"]
    pk[:, 49:177] = C["S16h"][0]
    pk[:, 177:305] = C["S16h"][1]
    pk[:, 305:433] = C["S16t"]
    dPack = nc.inline_tensor(pk, "cPack")
    # D stacked along columns: Dst[r, 256*k + j] = D_k[r, j]
    Dst = np.ascontiguousarray(C["Ds"].transpose(1, 0, 2).reshape(256, 1024))
    # EsT row-halves stacked along columns: EstA[c][p, 256k+j] = EsT[k][128c+p, j]
    EstA = [np.ascontiguousarray(
        C["EsT"][:, 128 * c:128 * (c + 1), :].transpose(1, 0, 2).reshape(
            128, 1024)) for c in range(2)]
    cvt = _to_bf16 if mm_bf16 else (lambda a: a)
    dDst = nc.inline_tensor(cvt(Dst), "cDst")
    dEstA = [nc.inline_tensor(cvt(EstA[c]), f"cEstA{c}") for c in range(2)]

    with tile.TileContext(nc) as tc:
        with (
            tc.tile_pool(name="cpool", bufs=1) as cpool,
            tc.tile_pool(name="abpool", bufs=1) as abpool,
            tc.tile_pool(name="wpool", bufs=2) as wpool,
            tc.tile_pool(name="qpool", bufs=2) as qpool,
            tc.tile_pool(name="ppmm", bufs=6, space="PSUM") as ppmm,
            tc.tile_pool(name="ppsm", bufs=2, space="PSUM") as ppsm,
        ):
            def cdma(dram_ap, shape, tag, dt_=F32):
                t = cpool.tile(shape, dt_, tag=tag, name=tag)
                if dt_ is F32R:
                    nc.sync.dma_start(t[:], dram_ap.bitcast(dt_))
                else:
                    nc.sync.dma_start(t[:], dram_ap)
                return t

            # dummy Sin on a memset scalar: enqueues the activation-table DMA
            # ahead of the constant DMAs so the table is loaded by the time
            # the real trig runs (the first matmul is gated on it)
            tdum = cpool.tile([1, 1], F32, tag="tdum", name="tdum")
            nc.vector.memset(tdum[:], 0.0)
            nc.scalar.activation(tdum[:], tdum[:], AF.Sin)

            # HAM pre-warm: the PE clock sits at 1.2GHz until ~3.4us of
            # sustained activity. The DMA/trig ramp leaves the PE idle for
            # ~4us at kernel start; fill it with independent matmuls on a
            # scratch tile so the real chain starts at 2.4GHz.
            tscr = cpool.tile([128, 512], BF16, tag="tscr", name="tscr")
            nc.vector.memset(tscr[:], 0.0)
            if N_WARMUP:
                ps_wu = ppmm.tile([128, 512], F32, tag="mm", name="wu")
                for i in range(N_WARMUP):
                    nc.tensor.matmul(ps_wu[:], tscr[:, 0:128], tscr[:],
                                     start=True, stop=True)
                # BIR verifier wants every PSUM write read back
                nc.vector.tensor_copy(tdum[:], ps_wu[0:1, 0:1])

            # -------- angle DMA first (critical path), then constants ------
            # one [1,128] angle tile: cols 0:96 = weight angles in the
            # CONTIGUOUS dram order (l, w, t) -- a strided dma pattern here
            # costs ~3us of descriptor drain, so the (t)-split is done with
            # strided SBUF views downstream instead. cols 96:128 = x.
            th = cpool.tile([1, 128], F32, tag="th", name="th")
            nc.sync.dma_start(th[:, 0:96],
                              wt[:].rearrange("l w t -> () (l w t)"))
            nc.sync.dma_start(th[:, 96:128],
                              xp[:].rearrange("b w -> () (b w)"))

            tPack = cdma(dPack[:, :], [16, 433], "pack")
            tN1c = tPack[0:1, 0:16]
            tE3 = tPack[0:4, 16:17]
            tS4 = tPack[0:4, 17:33]
            tS4t = tPack[0:4, 33:49]
            tS16h = [tPack[0:16, 49:177], tPack[0:16, 177:305]]
            tS16t = tPack[0:16, 305:433]
            # f32r view of the same constants (weights side of the fp32r
            # selector matmuls; DMA-sourced so walrus accepts the bitcast)
            tPackR = cdma(dPack[:, :], [16, 433], "packR", F32R)
            tN1cR = tPackR[0:1, 0:16]
            tE3R = tPackR[0:4, 16:17]
            tS4R = tPackR[0:4, 17:33]
            tS4tR = tPackR[0:4, 33:49]
            tS16hR = [tPackR[0:16, 49:177], tPackR[0:16, 177:305]]
            tS16tR = tPackR[0:16, 305:433]
            tDst = [cdma(dDst[128 * c:128 * (c + 1), :], [128, 1024],
                         f"dst{c}", mmdt) for c in range(2)]
            tEstA = [cdma(dEstA[c][:, :], [128, 1024], f"esa{c}", mmdt)
                     for c in range(2)]
            tEsT = [[tEstA[c][:, 256 * k:256 * (k + 1)] for c in range(2)]
                    for k in range(4)]

            tones = cpool.tile([128, 1], F32, tag="ones", name="ones")
            nc.vector.memset(tones[:], 1.0)
            tpi2 = cpool.tile([1, 1], F32, tag="pi2", name="pi2")
            nc.vector.memset(tpi2[:], HALF_PI)
            tone_row = cpool.tile([1, 48], F32, tag="tone_row", name="tone_row")
            nc.vector.memset(tone_row[:], 1.0)

            # ---------------- angles -> trig (weights + x together) --------
            sn = cpool.tile([1, 128], F32, tag="sn", name="sn")
            cs = cpool.tile([1, 128], F32, tag="cs", name="cs")
            nc.scalar.activation(sn[:], th[:], AF.Sin)
            nc.scalar.activation(cs[:], th[:], AF.Sin, bias=tpi2[:])

            # strided views splitting the interleaved (lw, t) angle order
            def tview(t, idx):
                return t[0:1, 0:96].rearrange(
                    "p (lw t) -> p lw t", t=2)[:, :, idx]

            sy, szr = tview(sn, 0), tview(sn, 1)
            cy, czr = tview(cs, 0), tview(cs, 1)
            xsin, xcos = sn[0:1, 96:128], cs[0:1, 96:128]
            # Rotblk rows as [1,192] vectors (r0..r3), then
            # F_all = sum_r N1[:, r] (x) row_r  via K=1 accumulating matmuls.
            # memsets first (no trig dep), then spread the row fills over
            # vector/scalar/gpsimd so the serial DVE chain shortens.
            # rv padded to 256 cols (zeros beyond 192) so the F-build matmuls
            # can run as fp32r with N>=256 (1 cyc/row vs fp32's 4); tiles are
            # f32r-typed so walrus sees explicitly-rounded matmul inputs
            # (memset can't write f32r directly — zero via fp32->f32r copy)
            zrow = cpool.tile([16, 256], F32, tag="zrow", name="zrow")
            nc.vector.memset(zrow[:], 0.0)
            rv = []
            for r in range(4):
                t = cpool.tile([1, 256], F32R, tag=f"rv{r}", name=f"rv{r}")
                nc.vector.tensor_copy(t[:], zrow[0:1, :])
                rv.append(t)
            rvv = [t[:, 0:192].rearrange("p (j n) -> p j n", n=4) for t in rv]

            def c3(a):
                return a.rearrange("p j -> p j ()")

            nc.vector.tensor_copy(rvv[0][:, :, 0:1], c3(tone_row[0:1, :]))

            pcc = cpool.tile([1, 48], F32, tag="pcc", name="pcc")  # cz*cy
            pcs = cpool.tile([1, 48], F32, tag="pcs", name="pcs")  # cz*sy
            psc = cpool.tile([1, 48], F32, tag="psc", name="psc")  # sz*cy
            pss = cpool.tile([1, 48], F32, tag="pss", name="pss")  # sz*sy
            nc.vector.tensor_mul(pcc[:], czr, cy)
            nc.vector.tensor_mul(pcs[:], czr, sy)
            nc.gpsimd.tensor_mul(psc[:], szr, cy)
            nc.gpsimd.tensor_mul(pss[:], szr, sy)

            nc.vector.tensor_copy(rvv[2][:, :, 2:3], c3(czr))
            nc.vector.tensor_copy(rvv[3][:, :, 3:4], c3(cy))
            nc.vector.tensor_copy(rvv[1][:, :, 1:2], c3(pcc[:]))
            nc.scalar.mul(rvv[1][:, :, 2:3], c3(szr), -1.0)
            nc.vector.tensor_copy(rvv[1][:, :, 3:4], c3(pcs[:]))
            nc.vector.tensor_copy(rvv[2][:, :, 1:2], c3(psc[:]))
            nc.vector.tensor_copy(rvv[2][:, :, 3:4], c3(pss[:]))
            nc.scalar.mul(rvv[3][:, :, 1:2], c3(sy), -1.0)

            ps_f = ppsm.tile([4, 256], F32, tag="sm", name="ps_f")
            for r in range(4):
                nc.tensor.matmul(ps_f[:], tN1cR[0:1, 4 * r:4 * (r + 1)],
                                 rv[r][:], start=(r == 0), stop=(r == 3))
            fall = cpool.tile([4, 256], F32R, tag="fall", name="fall")
            nc.vector.tensor_copy(fall[:], ps_f[:])

            # -------- layer-5 rank-1 start: arow = A(5)[192,:], brow = B(5)[0,:]
            # fall[m, 4*(8l+w)+n] = F(l,w)[m,n]; l=5 -> cols 160+4w..164+4w
            # DVE can't read from partition 3, so extract fall row 3 to
            # partition 0 with a K=4 selector matmul first.
            ps_f3 = ppsm.tile([1, 256], F32, tag="sm", name="ps_f3")
            nc.tensor.matmul(ps_f3[:], tE3R[:], fall[:], start=True, stop=True)
            fall3 = cpool.tile([1, 192], F32, tag="fall3", name="fall3")
            nc.vector.tensor_copy(fall3[:], ps_f3[:, 0:192])

            def fvec(w, row):
                o = 4 * (8 * 5 + w)
                if row == 3:
                    return fall3[0:1, o:o + 4]
                return fall[0:1, o:o + 4].bitcast(F32)

            def kron2(eng, dst16, va, vb):
                eng.tensor_mul(
                    dst16[:].rearrange("p (a b) -> p a b", a=4),
                    va.unsqueeze(2).broadcast_to([1, 4, 4]),
                    vb.unsqueeze(1).broadcast_to([1, 4, 4]))

            v01 = cpool.tile([1, 16], F32, tag="v01", name="v01")
            v23 = cpool.tile([1, 16], F32, tag="v23", name="v23")
            v45 = cpool.tile([1, 16], F32, tag="v45", name="v45")
            v67 = cpool.tile([1, 16], F32, tag="v67", name="v67")
            kron2(nc.vector, v01, fvec(0, 3), fvec(1, 0))
            kron2(nc.gpsimd, v23, fvec(2, 0), fvec(3, 0))
            kron2(nc.vector, v45, fvec(4, 0), fvec(5, 0))
            kron2(nc.gpsimd, v67, fvec(6, 0), fvec(7, 0))
            arow = cpool.tile([1, 256], mmdt, tag="arow", name="arow")
            brow = cpool.tile([1, 256], mmdt, tag="brow", name="brow")
            nc.vector.tensor_mul(
                arow[:].rearrange("p (a b) -> p a b", a=16),
                v01[:].unsqueeze(2).broadcast_to([1, 16, 16]),
                v23[:].unsqueeze(1).broadcast_to([1, 16, 16]))
            nc.gpsimd.tensor_mul(
                brow[:].rearrange("p (a b) -> p a b", a=16),
                v45[:].unsqueeze(2).broadcast_to([1, 16, 16]),
                v67[:].unsqueeze(1).broadcast_to([1, 16, 16]))

            # ------------- batched selector expansions -------------
            # t1a[p, 4j+n] = F_j[p>>2, n]; t2a[p, 4j+n] = F_j[p&3, n]
            ps1 = ppsm.tile([16, 256], F32, tag="sm", name="ps1")
            nc.tensor.matmul(ps1[:], tS4R[:], fall[:], start=True, stop=True)
            t1a = cpool.tile([16, 192], F32, tag="t1a", name="t1a")
            nc.scalar.copy(t1a[:], ps1[:, 0:192])
            ps2 = ppsm.tile([16, 256], F32, tag="sm", name="ps2")
            nc.tensor.matmul(ps2[:], tS4tR[:], fall[:], start=True, stop=True)
            t2a = cpool.tile([16, 192], F32, tag="t2a", name="t2a")
            nc.scalar.copy(t2a[:], ps2[:, 0:192])

            # pair-kron tiles for all layers: fpa[pos][p, 16l + 4a+b]
            # (built on GpSimd to keep the DVE free for PSUM copies)
            fpa = []
            for pos in range(4):
                fp = abpool.tile([16, 256], F32R, tag=f"fpa{pos}",
                                 name=f"fpa{pos}")
                nc.vector.tensor_copy(fp[:, 96:256], zrow[:, 96:256])
                for l in range(DEPTH):
                    o = 32 * l + 8 * pos
                    nc.vector.tensor_mul(
                        fp[:, 16 * l:16 * (l + 1)].rearrange(
                            "p (a b) -> p a b", a=4),
                        t1a[:, o:o + 4].unsqueeze(2).broadcast_to([16, 4, 4]),
                        t2a[:, o + 4:o + 8].unsqueeze(1).broadcast_to([16, 4, 4]),
                    )
                fpa.append(fp)

            # quad selector expansions, batched over layers: [128, 96]
            # (fp32r with N=256 padding: 1 cyc/row on the PE vs fp32's 4)
            def sel_expand(sel, fp_all, tag):
                ps = ppsm.tile([128, 256], F32, tag="sm", name=f"ps{tag}")
                nc.tensor.matmul(ps[:], sel[:], fp_all[:],
                                 start=True, stop=True)
                t = cpool.tile([128, 96], F32, tag=tag, name=tag)
                nc.scalar.copy(t[:], ps[:, 0:96])
                return t

            zA = [sel_expand(tS16hR[c], fpa[0], f"zA{c}") for c in range(2)]
            yA = sel_expand(tS16tR, fpa[1], "yA")
            zB = [sel_expand(tS16hR[c], fpa[2], f"zB{c}") for c in range(2)]
            yB = sel_expand(tS16tR, fpa[3], "yB")

            # per-layer A/B kron tile build (GpSimd, SBUF->SBUF), called
            # lazily mid-chain; keeps DVE/ACT free for PSUM copies
            def build_ab(l):
                sl = slice(16 * l, 16 * (l + 1))
                Al, Bl = [], []
                for c in range(2):
                    ab = abpool.tile([128, 256], mmdt, tag=f"A{l}_{c}",
                                     name=f"A{l}_{c}")
                    nc.vector.tensor_mul(
                        ab[:].rearrange("p (a b) -> p a b", a=16),
                        zA[c][:, sl].unsqueeze(2).broadcast_to([128, 16, 16]),
                        yA[:, sl].unsqueeze(1).broadcast_to([128, 16, 16]),
                    )
                    Al.append(ab)
                    bb = abpool.tile([128, 256], mmdt, tag=f"B{l}_{c}",
                                     name=f"B{l}_{c}")
                    nc.gpsimd.tensor_mul(
                        bb[:].rearrange("p (a b) -> p a b", a=16),
                        zB[c][:, sl].unsqueeze(2).broadcast_to([128, 16, 16]),
                        yB[:, sl].unsqueeze(1).broadcast_to([128, 16, 16]),
                    )
                    Bl.append(bb)
                return Al, Bl

            # ---------------- the chain ----------------
            # PSUM->SBUF copies alternate Scalar/Vector per psum tile (the
            # two m-halves live in different banks, so the engines overlap).
            copy_engines = [nc.scalar.copy,
                            nc.vector.tensor_copy]
            copy_flip = [0]

            def copy_piece(dst, src):
                copy_engines[copy_flip[0] % len(copy_engines)](dst, src)
                copy_flip[0] += 1

            # ---- low-rank phase: the pullback has rank 1 at the layer-5
            # start and rank 4^s after s layers, so layers 5/4/3 run on
            # rank-4/16/64 factors (q = L R^T) with tiny-N matmuls instead
            # of dense 256x256 sandwiches. All stationary operands are the
            # same At/Bt/tEstA/tDst tiles the dense chain uses.
            #
            # seed: W(5) = a b^T (a = arow^T, b = brow^T) =>
            #   L0 = [E_k a], R0 = [D_k^T b]  (rank 4)
            onebf = cpool.tile([1, 1], mmdt, tag="onebf", name="onebf")
            nc.vector.memset(onebf[:], 1.0)
            pcol = ppsm.tile([128, 4], F32, tag="sm", name="pcol")
            for ci, srow in enumerate((arow, brow)):
                for c in range(2):
                    nc.tensor.matmul(pcol[:, 2 * ci + c:2 * ci + c + 1],
                                     srow[0:1, 128 * c:128 * (c + 1)],
                                     onebf[:], start=True, stop=True,
                                     skip_group_check=True)
            abcol = cpool.tile([128, 4], mmdt, tag="abcol", name="abcol")
            nc.vector.tensor_copy(abcol[:], pcol[:])
            acol = [abcol[:, 0:1], abcol[:, 1:2]]
            bcol = [abcol[:, 2:3], abcol[:, 3:4]]

            # A/B tiles for the factored layers (emitted early: DVE builds
            # them under the seed's PE work)
            At4, Bt4 = build_ab(4)
            At3, Bt3 = build_ab(3)

            L0, R0 = [], []
            for m in range(2):
                psL = ppmm.tile([128, 4], F32, tag="mm", name=f"psL0{m}")
                psR = ppmm.tile([128, 4], F32, tag="mm", name=f"psR0{m}")
                for k in range(4):
                    for c in range(2):
                        nc.tensor.matmul(
                            psL[:, k:k + 1],
                            tEstA[c][:, 256 * k + 128 * m:
                                     256 * k + 128 * (m + 1)],
                            acol[c], start=(c == 0), stop=(c == 1),
                            skip_group_check=True)
                        nc.tensor.matmul(
                            psR[:, k:k + 1],
                            tDst[c][:, 256 * k + 128 * m:
                                    256 * k + 128 * (m + 1)],
                            bcol[c], start=(c == 0), stop=(c == 1),
                            skip_group_check=True)
                tL = wpool.tile([128, 4], mmdt, tag=f"L0{m}", name=f"L0{m}")
                tR = wpool.tile([128, 4], mmdt, tag=f"R0{m}", name=f"R0{m}")
                nc.scalar.copy(tL[:], psL[:])
                nc.vector.tensor_copy(tR[:], psR[:])
                L0.append(tL)
                R0.append(tR)

            # one factored layer: (L, R, r) -> ([E_k A L], [D_k^T B^T R])
            def factored_layer(Lf, Rf, Atl, Btl, r, tagp):
                XL, XR = [], []
                for m in range(2):
                    psX = ppmm.tile([128, r], F32, tag="mm", name=f"psXL{m}")
                    psY = ppmm.tile([128, r], F32, tag="mm", name=f"psXR{m}")
                    for c in range(2):
                        nc.tensor.matmul(
                            psX[:], Atl[c][:, 128 * m:128 * (m + 1)],
                            Lf[c][:], start=(c == 0), stop=(c == 1))
                        nc.tensor.matmul(
                            psY[:], Btl[c][:, 128 * m:128 * (m + 1)],
                            Rf[c][:], start=(c == 0), stop=(c == 1))
                    tX = wpool.tile([128, r], mmdt, tag=f"XL{tagp}{m}",
                                    name=f"XL{tagp}{m}")
                    tY = wpool.tile([128, r], mmdt, tag=f"XR{tagp}{m}",
                                    name=f"XR{tagp}{m}")
                    nc.scalar.copy(tX[:], psX[:])
                    nc.vector.tensor_copy(tY[:], psY[:])
                    XL.append(tX)
                    XR.append(tY)
                Lout, Rout = [], []
                for m in range(2):
                    psL = ppmm.tile([128, 4 * r], F32, tag="mm",
                                    name=f"psL{tagp}{m}")
                    psR = ppmm.tile([128, 4 * r], F32, tag="mm",
                                    name=f"psR{tagp}{m}")
                    for k in range(4):
                        for c in range(2):
                            nc.tensor.matmul(
                                psL[:, r * k:r * (k + 1)],
                                tEstA[c][:, 256 * k + 128 * m:
                                         256 * k + 128 * (m + 1)],
                                XL[c][:], start=(c == 0), stop=(c == 1),
                                skip_group_check=True)
                            nc.tensor.matmul(
                                psR[:, r * k:r * (k + 1)],
                                tDst[c][:, 256 * k + 128 * m:
                                        256 * k + 128 * (m + 1)],
                                XR[c][:], start=(c == 0), stop=(c == 1),
                                skip_group_check=True)
                    tL = wpool.tile([128, 4 * r], mmdt, tag=f"Lo{tagp}{m}",
                                    name=f"Lo{tagp}{m}")
                    tR = wpool.tile([128, 4 * r], mmdt, tag=f"Ro{tagp}{m}",
                                    name=f"Ro{tagp}{m}")
                    nc.scalar.copy(tL[:], psL[:])
                    nc.vector.tensor_copy(tR[:], psR[:])
                    Lout.append(tL)
                    Rout.append(tR)
                return Lout, Rout

            # layer 4 on rank-4 factors -> rank 16
            L1, R1 = factored_layer(L0, R0, At4, Bt4, 4, "f1")

            # layer 3: rotate (A L1 / B^T R1 at r=16), then produce the
            # E/D-expanded factors TRANSPOSED ([64, 256], k-blocks at
            # partition 32k) so the rank-64 q3 materializes directly.
            At2, Bt2 = build_ab(2)
            XL2, XR2 = [], []
            for m in range(2):
                psX = ppmm.tile([128, 16], F32, tag="mm", name=f"psXL2{m}")
                psY = ppmm.tile([128, 16], F32, tag="mm", name=f"psXR2{m}")
                for c in range(2):
                    nc.tensor.matmul(psX[:], At3[c][:, 128 * m:128 * (m + 1)],
                                     L1[c][:], start=(c == 0), stop=(c == 1))
                    nc.tensor.matmul(psY[:], Bt3[c][:, 128 * m:128 * (m + 1)],
                                     R1[c][:], start=(c == 0), stop=(c == 1))
                tX = wpool.tile([128, 16], mmdt, tag=f"XL2{m}",
                                name=f"XL2{m}")
                tY = wpool.tile([128, 16], mmdt, tag=f"XR2{m}",
                                name=f"XR2{m}")
                nc.scalar.copy(tX[:], psX[:])
                nc.vector.tensor_copy(tY[:], psY[:])
                XL2.append(tX)
                XR2.append(tY)
            # (partition offsets are limited to {0, 32, 64} and quadrant 3
            # is unusable, so the 4 k-blocks go 2-per-tile at offsets 0/32)
            LTs, RTs = [], []
            for h in range(2):
                psLT = ppmm.tile([64, 256], F32, tag="mm", name=f"psLT{h}")
                psRT = ppmm.tile([64, 256], F32, tag="mm", name=f"psRT{h}")
                for kk in range(2):
                    k = 2 * h + kk
                    for c in range(2):
                        nc.tensor.matmul(
                            psLT[32 * kk:32 * kk + 16, :],
                            XL2[c][:], tEstA[c][:, 256 * k:256 * (k + 1)],
                            start=(c == 0), stop=(c == 1),
                            skip_group_check=True)
                        nc.tensor.matmul(
                            psRT[32 * kk:32 * kk + 16, :],
                            XR2[c][:], tDst[c][:, 256 * k:256 * (k + 1)],
                            start=(c == 0), stop=(c == 1),
                            skip_group_check=True)
                LT = wpool.tile([64, 256], mmdt, tag=f"LT{h}", name=f"LT{h}")
                RT = wpool.tile([64, 256], mmdt, tag=f"RT{h}", name=f"RT{h}")
                for kk in range(2):
                    copy_piece(LT[32 * kk:32 * kk + 16, :],
                               psLT[32 * kk:32 * kk + 16, :])
                    copy_piece(RT[32 * kk:32 * kk + 16, :],
                               psRT[32 * kk:32 * kk + 16, :])
                LTs.append(LT)
                RTs.append(RT)

            # materialize q3 = sum_k (E_k A L1')(D_k^T B^T R1')^T
            q_sb = []
            for m in range(2):
                ps = ppmm.tile([128, 256], F32, tag="mm", name=f"ps_q3{m}")
                for h in range(2):
                    for kk in range(2):
                        nc.tensor.matmul(
                            ps[:], LTs[h][32 * kk:32 * kk + 16,
                                          128 * m:128 * (m + 1)],
                            RTs[h][32 * kk:32 * kk + 16, :],
                            start=(h == 0 and kk == 0),
                            stop=(h == 1 and kk == 1))
                t = qpool.tile([128, 256], mmdt, tag=f"q{m}", name=f"q{m}")
                copy_piece(t[:], ps[:])
                q_sb.append(t)

            At, Bt = At2, Bt2
            At_next = Bt_next = None
            for s in range(3, DEPTH):
                l = DEPTH - 1 - s
                # emit next layer's A/B tile build first so the DVE runs it
                # under this layer's matmuls (no dep on the chain)
                if l > 0:
                    At_next, Bt_next = build_ab(l - 1)
                if True:
                    # Tp = q^T @ A   [C, R']
                    tp_sb = []
                    for m in range(2):
                        ps = ppmm.tile([128, 256], F32, tag="mm", name="ps_tp")
                        for c in range(2):
                            nc.tensor.matmul(
                                ps[:], q_sb[c][:, 128 * m:128 * (m + 1)],
                                At[c][:], start=(c == 0), stop=(c == 1))
                        t = wpool.tile([128, 256], mmdt, tag=f"tp{m}",
                                       name=f"tp{m}")
                        copy_piece(t[:], ps[:])
                        tp_sb.append(t)
                    # Wp = B^T @ Tp  [C', R']
                    wp_sb = []
                    for m in range(2):
                        ps = ppmm.tile([128, 256], F32, tag="mm", name="ps_wp")
                        for c in range(2):
                            nc.tensor.matmul(
                                ps[:], Bt[c][:, 128 * m:128 * (m + 1)],
                                tp_sb[c][:], start=(c == 0), stop=(c == 1))
                        t = wpool.tile([128, 256], mmdt, tag=f"wp{m}",
                                       name=f"wp{m}")
                        copy_piece(t[:], ps[:])
                        wp_sb.append(t)
                # U = W @ [D_0|D_1|D_2|D_3]   [R', (k,j)] as [128, 1024] tiles
                uall = []
                for m in range(2):
                    u = wpool.tile([128, 1024], mmdt, tag=f"u{m}", name=f"u{m}")
                    for nh in range(2):
                        ps = ppmm.tile([128, 512], F32, tag="mm", name="ps_u")
                        for c in range(2):
                            nc.tensor.matmul(
                                ps[:], wp_sb[c][:, 128 * m:128 * (m + 1)],
                                tDst[c][:, 512 * nh:512 * (nh + 1)],
                                start=(c == 0), stop=(c == 1))
                        copy_piece(u[:, 512 * nh:512 * (nh + 1)], ps[:])
                    uall.append(u)
                # q' = sum_k E_k U_k
                q_new = []
                for m in range(2):
                    ps = ppmm.tile([128, 256], F32, tag="mm", name="ps_q")
                    first = True
                    for c in range(2):
                        for k in range(4):
                            nc.tensor.matmul(
                                ps[:], tEsT[k][c][:, 128 * m:128 * (m + 1)],
                                uall[c][:, 256 * k:256 * (k + 1)],
                                start=first, stop=(c == 1 and k == 3))
                            first = False
                    t = qpool.tile([128, 256], mmdt, tag=f"q{m}", name=f"q{m}")
                    copy_piece(t[:], ps[:])
                    q_new.append(t)
                q_sb = q_new
                At, Bt = At_next, Bt_next
                if l == 2:
                    # ---------------- encoding vectors (for the finale) ----
                    ones32 = cpool.tile([1, 32], F32, tag="ones32",
                                        name="ones32")
                    nc.vector.memset(ones32[:], 1.0)
                    ps_e = ppsm.tile([4, 32], F32, tag="sm", name="ps_e")
                    for i, (r, src_row) in enumerate(
                            [(0, ones32[:]), (1, xsin), (3, xcos)]):
                        nc.tensor.matmul(ps_e[:],
                                         tN1c[0:1, 4 * r:4 * (r + 1)],
                                         src_row, start=(i == 0), stop=(i == 2))
                    aenc = cpool.tile([4, 32], F32, tag="aenc", name="aenc")
                    nc.vector.tensor_copy(aenc[:], ps_e[:])

                    pse1 = ppsm.tile([16, 32], F32, tag="sm", name="pse1")
                    nc.tensor.matmul(pse1[:], tS4[:], aenc[:],
                                     start=True, stop=True)
                    s1e = cpool.tile([16, 32], F32, tag="s1e", name="s1e")
                    nc.vector.tensor_copy(s1e[:], pse1[:])
                    pse2 = ppsm.tile([16, 32], F32, tag="sm", name="pse2")
                    nc.tensor.matmul(pse2[:], tS4t[:], aenc[:],
                                     start=True, stop=True)
                    s2e = cpool.tile([16, 32], F32, tag="s2e", name="s2e")
                    nc.vector.tensor_copy(s2e[:], pse2[:])

                    def wcol(t, w):
                        return t[:].rearrange("p (b w) -> p b w", w=8)[:, :, w]

                    # ahi = [a01 | a45], alo = [a23 | a67] (cols = 4 samples)
                    ahi = cpool.tile([16, 8], F32, tag="ahi", name="ahi")
                    alo = cpool.tile([16, 8], F32, tag="alo", name="alo")
                    nc.vector.tensor_mul(ahi[:, 0:4], wcol(s1e, 0), wcol(s2e, 1))
                    nc.vector.tensor_mul(ahi[:, 4:8], wcol(s1e, 4), wcol(s2e, 5))
                    nc.vector.tensor_mul(alo[:, 0:4], wcol(s1e, 2), wcol(s2e, 3))
                    nc.vector.tensor_mul(alo[:, 4:8], wcol(s1e, 6), wcol(s2e, 7))

                    psy = ppsm.tile([128, 8], F32, tag="sm", name="psy")
                    nc.tensor.matmul(psy[:], tS16t[:], alo[:],
                                     start=True, stop=True)
                    yq = cpool.tile([128, 8], F32, tag="yq", name="yq")
                    nc.vector.tensor_copy(yq[:], psy[:])
                    Pr = []
                    Pc = []
                    for c in range(2):
                        psz = ppsm.tile([128, 8], F32, tag="sm", name="psz")
                        nc.tensor.matmul(psz[:], tS16h[c][:], ahi[:],
                                         start=True, stop=True)
                        pr = cpool.tile([128, B_PER], mmdt, tag=f"pr{c}",
                                        name=f"pr{c}")
                        nc.vector.tensor_mul(pr[:], psz[:, 0:4], yq[:, 0:4])
                        pc = cpool.tile([128, B_PER], F32, tag=f"pc{c}",
                                        name=f"pc{c}")
                        nc.vector.tensor_mul(pc[:], psz[:, 4:8], yq[:, 4:8])
                        Pr.append(pr)
                        Pc.append(pc)

            # ---------------- final contraction ----------------
            h_sb = []
            for m in range(2):
                ps = ppsm.tile([128, B_PER], F32, tag="sm", name="ps_g")
                for c in range(2):
                    nc.tensor.matmul(
                        ps[:], q_sb[c][:, 128 * m:128 * (m + 1)],
                        Pr[c][:], start=(c == 0), stop=(c == 1))
                h = cpool.tile([128, B_PER], F32, tag=f"h{m}", name=f"h{m}")
                nc.vector.tensor_mul(h[:], ps[:], Pc[m][:])
                h_sb.append(h)
            ps_o = ppsm.tile([B_PER, 1], F32, tag="sm", name="ps_o")
            for m in range(2):
                nc.tensor.matmul(ps_o[:], h_sb[m][:], tones[:],
                                 start=(m == 0), stop=(m == 1))
            out_sb = cpool.tile([B_PER, 1], F32, tag="osb", name="osb")
            nc.vector.tensor_copy(out_sb[:], ps_o[:])
            nc.sync.dma_start(out_d[:, :], out_sb[:])

    nc.compile()
    return nc


# ---------------------------------------------------------------------------
# Host entry point
# ---------------------------------------------------------------------------

_NC = None


def _get_nc():
    global _NC
    if _NC is None:
        _NC = build_nc(mm_bf16=os.environ.get("QK_MM_F32R") != "1")
    return _NC


def _maybe_enable_ldw_opt():
    if os.environ.get("QK_LDW_OPT") != "1":
        return
    from concourse.compiler_utils import get_compiler_flags, set_compiler_flags

    flags = [f.replace("--enable-ldw-opt=false", "--enable-ldw-opt=true")
             for f in get_compiler_flags()]
    set_compiler_flags(flags)


def kernel(x: np.ndarray, weights: np.ndarray) -> np.ndarray:
    from concourse.bass_utils import run_bass_kernel_spmd

    _maybe_enable_ldw_opt()

    nc = _get_nc()
    x = np.ascontiguousarray(x, dtype=np.float32)
    weights = np.ascontiguousarray(weights, dtype=np.float32)
    in_maps = [
        {"xp": x[i * B_PER:(i + 1) * B_PER], "wt": weights}
        for i in range(N_CORES)
    ]
    res = run_bass_kernel_spmd(nc, in_maps, list(range(N_CORES)))
    out = np.concatenate([res.results[i]["out"] for i in range(N_CORES)], axis=0)
    return out.astype(np.float32)

